# revision 7
# baseline (speedup 1.0000x reference)
"""Multi-head attention block on 8 TRN2 NeuronCores.

Problem: x[2,2048,768] -> qkv proj -> 12-head attention -> out proj.
Sharding: 24 (batch, head) pairs across 8 cores; core c handles batch
c//4 and heads 3*(c%4)..3*(c%4)+2. Each core computes its heads'
Q,K,V, attention, and a partial output projection; the host sums the
four per-batch partials and adds the bias terms.

v3 design notes:
  - All matmul operands are bf16 (1 cycle/row on the PE at any free
    size); PSUM accumulation stays f32; DRAM in/out tensors are bf16
    (host converts), halving DMA bytes vs f32.
  - Phase B is ACT-bound: 96 exps of [128,1024] (~1.05us effective
    spacing) are the kernel's pace-setter, so ACT does nothing else.
    Phase A work is spread: Q+K PSUM copies on DVE, V copies on
    gpsimd, pad zero-fills/ones via memsets (no DMA). The Exp table
    is preloaded via a dummy activation during the DMA wait.
  - Input DMA is split across two hardware queues (sync + gpsimd) in
    critical-first order so the first projection matmul waits only on
    ~1MB and phase A is never DMA-starved.
  - Phase A: per 512-column slab of x^T, one fused Q+K projection
    (3 PSUM groups of 128 = 384 rows; wq pre-scaled by 0.125
    host-side) plus keys-major V (192 free) for 4 key chunks.
  - Attention: S -> exp -> AV via a global pend queue (lag 2);
    denominators from a ones column appended to V (AV PSUM row 64);
    normalize = approx-reciprocal straight off the PSUM denominator
    row + one DVE multiply with a 0-stride partition-broadcast
    operand (no gpsimd broadcast, no sums staging).
  - Output projection: dedicated 2-bank PSUM tag; block 0's eight
    chunks ride the ACT-paced PE slack one-at-a-time every other
    m-round early in block 1 (DVE casts); block 1's chunks form the
    tail, alternating between the proj tag and the then-free S tag,
    with PSUM->SBUF casts on the then-idle ACT engine.
  - k-bias dropped (softmax shift invariance); v-bias and proj-bias
    fold in on the host: out += b_proj + b_v @ w_proj; q-bias adds on
    the DVE Q-copy (scaled 0.125 host-side like wq).
"""

import os
import sys

for _p in ("/opt/trn_rl_repo", "/opt/pypackages"):
    if _p not in sys.path:
        sys.path.append(_p)

import numpy as np

B, N, C = 2, 2048, 768
H, D = 12, 64
HPC = 3                    # heads per core
J = HPC * D                # 192 per-core head-dim rows
NCORES = 8
NBLK = 1024                # query-block width (one exp per [128, NBLK])
NB = N // NBLK             # 2
MC = N // 128              # 16 key chunks
KC = C // 128              # 6 contraction chunks for projections
NSLAB = 4                  # x^T column slabs of 512

_cache = {}
LAST_RESULTS = None


def _build():
    import concourse.mybir as mybir
    import concourse.tile as tile
    from concourse import bacc

    f32 = mybir.dt.float32
    bf16 = mybir.dt.bfloat16
    Exp = mybir.ActivationFunctionType.Exp
    Copy = mybir.ActivationFunctionType.Copy
    mult = mybir.AluOpType.mult
    add = mybir.AluOpType.add

    nc = bacc.Bacc("TRN2", target_bir_lowering=False, debug=False,
                   num_devices=NCORES)

    xt_d = nc.declare_dram_parameter("xt", [C, N], bf16, isOutput=False)
    wqk_d = nc.declare_dram_parameter("wqk", [C, 2 * J], bf16,
                                      isOutput=False)
    wv_d = nc.declare_dram_parameter("wv", [C, J], bf16, isOutput=False)
    bq_d = nc.declare_dram_parameter("bq", [J, 1], f32, isOutput=False)
    wp_d = nc.declare_dram_parameter("wp", [2 * 128, C], bf16,
                                     isOutput=False)
    out_d = nc.declare_dram_parameter("out", [N, C], bf16, isOutput=True)

    with tile.TileContext(nc) as tc:
        with (
            tc.tile_pool(name="persist", bufs=1) as pp,
            tc.tile_pool(name="osb", bufs=4) as posb,
        ):
            bqt = [pp.tile([64, 1], f32, tag=f"bq{h}", name=f"bq{h}")
                   for h in range(HPC)]
            xt = [pp.tile([128, N], bf16, tag=f"xt{i}", name=f"xt{i}")
                  for i in range(KC)]
            wqk = [pp.tile([128, 2 * J], bf16, tag=f"wqk{i}", name=f"wqk{i}")
                   for i in range(KC)]
            wv = [pp.tile([128, J], bf16, tag=f"wv{i}", name=f"wv{i}")
                  for i in range(KC)]
            # padded Q^T/K^T per head: rows 0:64 data, rows 64:128 zero
            qh = [pp.tile([128, N], bf16, tag=f"qh{h}", name=f"qh{h}")
                  for h in range(HPC)]
            kh = [pp.tile([128, N], bf16, tag=f"kh{h}", name=f"kh{h}")
                  for h in range(HPC)]
            # V keys-major with a ones column per head: [128, 3*65]
            vx = [pp.tile([128, HPC * 65], bf16, tag=f"vx{m}",
                          name=f"vx{m}") for m in range(MC)]
            wp = [pp.tile([128, C], bf16, tag=f"wp{t}", name=f"wp{t}")
                  for t in range(2)]
            # normalized attention outputs: ah2[0] = heads 0,1;
            # ah2[1] rows 0:64 = head 2, rows 64:128 zero
            ah2 = [pp.tile([128, N], bf16, tag=f"ah2{t}", name=f"ah2{t}")
                   for t in range(2)]
            dummy = pp.tile([1, 4], f32, tag="dummy", name="dummy")

            # gpsimd DMA queue: wv, xt slabs 2-3, wp (issued first so
            # the queue starts concurrently with sync's)
            for i in range(KC):
                nc.gpsimd.dma_start(wv[i][:], wv_d[128 * i:128 * (i + 1), :])
            for s in (2, 3):
                nsl = slice(512 * s, 512 * (s + 1))
                for i in range(KC):
                    nc.gpsimd.dma_start(xt[i][:, nsl],
                                        xt_d[128 * i:128 * (i + 1), nsl])
            for t in range(2):
                nc.gpsimd.dma_start(wp[t][:],
                                    wp_d[128 * t:128 * (t + 1), :])
            # sync DMA queue: bq, then wqk/xt-slab0 interleaved, slab1
            for h in range(HPC):
                nc.sync.dma_start(bqt[h][:], bq_d[64 * h:64 * (h + 1), :])
            for i in range(KC):
                nc.sync.dma_start(wqk[i][:],
                                  wqk_d[128 * i:128 * (i + 1), :])
                nc.sync.dma_start(xt[i][:, 0:512],
                                  xt_d[128 * i:128 * (i + 1), 0:512])
            for i in range(KC):
                nc.sync.dma_start(xt[i][:, 512:1024],
                                  xt_d[128 * i:128 * (i + 1), 512:1024])

            # zero-fill pads and ones columns; Exp table preload on ACT
            nc.vector.memset(dummy[:], 0.0)
            nc.scalar.activation(dummy[:, 2:4], dummy[:, 0:2], Exp)
            for h in range(HPC):
                nc.vector.memset(qh[h][64:128, :], 0.0)
                nc.gpsimd.memset(kh[h][64:128, :], 0.0)
            nc.vector.memset(ah2[1][64:128, :], 0.0)
            for m in range(MC):
                on = vx[m].rearrange("p (h e) -> p h e", e=65)[:, :, 64:65]
                nc.gpsimd.memset(on, 1.0)

            # ---- Phase A: fused Q+K projection + V, slab by slab ----
            with tc.tile_pool(name="ps1", bufs=2, space="PSUM") as ps1:
                for s in range(NSLAB):
                    nsl = slice(512 * s, 512 * (s + 1))
                    for g in range(3):
                        gsl = slice(128 * g, 128 * (g + 1))
                        ps = ps1.tile([128, 512], f32, tag="qk", bufs=3,
                                      name="ps_qk")
                        for k in range(KC):
                            nc.tensor.matmul(
                                ps[:], wqk[k][:, gsl], xt[k][:, nsl],
                                start=(k == 0), stop=(k == KC - 1))
                        for half in range(2):
                            idx = 2 * g + half
                            src = ps[64 * half:64 * (half + 1), :]
                            if idx < HPC:
                                nc.vector.tensor_scalar(
                                    qh[idx][0:64, nsl], src, 1.0,
                                    bqt[idx][:], mult, add)
                            else:
                                nc.vector.tensor_copy(
                                    kh[idx - HPC][0:64, nsl], src)
                    for m in range(4 * s, 4 * s + 4):
                        msl = slice(128 * m, 128 * (m + 1))
                        psv = ps1.tile([128, J], f32, tag="v", bufs=2,
                                       name="ps_v")
                        for k in range(KC):
                            nc.tensor.matmul(psv[:], xt[k][:, msl], wv[k][:],
                                             start=(k == 0),
                                             stop=(k == KC - 1))
                        vdst = vx[m].rearrange("p (h e) -> p h e",
                                               e=65)[:, :, 0:64]
                        nc.vector.tensor_copy(
                            vdst, psv.rearrange("p (h e) -> p h e", e=64))

            # ---- Phase B: attention + interleaved projection ----
            with (
                tc.tile_pool(name="etile", bufs=4) as pe,
                tc.tile_pool(name="bcsb", bufs=4) as pbc,
                tc.tile_pool(name="ps2", bufs=1, space="PSUM") as ps2,
            ):
                pend = []

                def flush_one():
                    avh, h, nb, mm, ee = pend.pop(0)
                    vsl = slice(65 * h, 65 * (h + 1))
                    for i in range(NBLK // 512):
                        nc.tensor.matmul(
                            avh[i][:], vx[mm][:, vsl],
                            ee[:, 512 * i:512 * (i + 1)],
                            start=(mm == 0), stop=(mm == MC - 1))
                    if mm != MC - 1:
                        return
                    # normalize: approx-reciprocal of the PSUM
                    # denominator row, broadcast via 0-stride operand
                    adst, r0 = ((ah2[0], 0) if h == 0 else
                                (ah2[0], 64) if h == 1 else
                                (ah2[1], 0))
                    for i in range(NBLK // 512):
                        hf = slice(NBLK * nb + 512 * i,
                                   NBLK * nb + 512 * (i + 1))
                        dn = pbc.tile([1, 512], f32, tag="dn", name="dn")
                        nc.vector.tensor_copy(dn[:], avh[i][64:65, :])
                        rec = pbc.tile([1, 512], f32, tag="rec",
                                       name="rec")
                        nc.vector.reciprocal_approx_fast(rec[:], dn[:])
                        bcs = pbc.tile([64, 512], f32, tag="bcs",
                                       name="bcs")
                        nc.gpsimd.partition_broadcast(bcs[:], rec[:])
                        nc.vector.tensor_mul(
                            adst[r0:r0 + 64, hf], avh[i][0:64, :], bcs[:])

                def proj_chunk(mi, tag, tail):
                    msl = slice(128 * mi, 128 * (mi + 1))
                    if tag == "s":
                        pj = ps2.tile([128, NBLK], f32, tag="s", bufs=2,
                                      name="ps_pj")
                    else:
                        pj = ps2.tile([128, C], f32, tag="pj", bufs=1,
                                      name="ps_pj")
                    for f0, fn in ((0, 512), (512, 256)):
                        for t in range(2):
                            nc.tensor.matmul(
                                pj[:, f0:f0 + fn], ah2[t][:, msl],
                                wp[t][:, f0:f0 + fn],
                                start=(t == 0), stop=(t == 1))
                    o3 = posb.tile([128, C], bf16, tag="o3", name="o3")
                    if tail:
                        nc.scalar.activation(o3[:], pj[:, 0:C], Copy)
                    else:
                        nc.vector.tensor_copy(o3[:], pj[:, 0:C])
                    nc.sync.dma_start(out_d[msl, :], o3[:])

                for nb in range(NB):
                    for h in range(HPC):
                        avh = [ps2.tile([65, 512], f32, tag=f"av{i}",
                                        bufs=1, name=f"ps_av{i}")
                               for i in range(NBLK // 512)]
                        for m in range(MC):
                            msl = slice(128 * m, 128 * (m + 1))
                            s = ps2.tile([128, NBLK], f32, tag="s",
                                         bufs=2, name="ps_s")
                            for i in range(NBLK // 512):
                                nc.tensor.matmul(
                                    s[:, 512 * i:512 * (i + 1)],
                                    kh[h][:, msl],
                                    qh[h][:, NBLK * nb + 512 * i:
                                          NBLK * nb + 512 * (i + 1)])
                            e = pe.tile([128, NBLK], bf16, tag="e",
                                        name="e")
                            nc.scalar.activation(e[:], s[:], Exp)
                            pend.append((avh, h, nb, m, e))
                            if len(pend) > 2:
                                flush_one()
                            # block-0 projection rides the ACT-paced PE
                            # slack, one chunk every other m-round
                            if nb == 1 and h == 0 and m >= 2 and m % 2 == 0:
                                proj_chunk(m // 2 - 1, "pj", False)
                            if nb == 1 and h == 1 and m == 2:
                                proj_chunk(7, "pj", False)
                while pend:
                    flush_one()
                for mi in range(8, 16):
                    proj_chunk(mi, "s" if mi % 2 else "pj", True)

    nc.compile()
    return nc


def kernel(x, w_qkv, b_qkv, w_proj, b_proj):
    import ml_dtypes
    from concourse.bass_utils import run_bass_kernel_spmd

    global LAST_RESULTS
    if "nc" not in _cache:
        _cache["nc"] = _build()
    nc = _cache["nc"]

    bf = ml_dtypes.bfloat16
    x = np.asarray(x, dtype=np.float32)
    w_qkv = np.asarray(w_qkv, dtype=np.float32)
    b_qkv = np.asarray(b_qkv, dtype=np.float32)
    w_proj = np.asarray(w_proj, dtype=np.float32)
    b_proj = np.asarray(b_proj, dtype=np.float32)

    in_maps = []
    for c in range(NCORES):
        b = c // 4
        h0 = HPC * (c % 4)
        cs = slice(64 * h0, 64 * (h0 + HPC))
        ks = slice(C + 64 * h0, C + 64 * (h0 + HPC))
        vs = slice(2 * C + 64 * h0, 2 * C + 64 * (h0 + HPC))
        wqk_cat = np.concatenate(
            [w_qkv[:, cs] * 0.125, w_qkv[:, ks]], axis=1)
        wp_pad = np.zeros((2 * 128, C), dtype=np.float32)
        wp_pad[0:128] = w_proj[64 * h0:64 * (h0 + 2), :]
        wp_pad[128:192] = w_proj[64 * (h0 + 2):64 * (h0 + 3), :]
        in_maps.append({
            "xt": np.ascontiguousarray(x[b].T).astype(bf),
            "wqk": wqk_cat.astype(bf),
            "wv": np.ascontiguousarray(w_qkv[:, vs]).astype(bf),
            "bq": np.ascontiguousarray(
                (b_qkv[cs] * 0.125).reshape(J, 1)),
            "wp": wp_pad.astype(bf),
        })

    res = run_bass_kernel_spmd(nc, in_maps, core_ids=list(range(NCORES)))
    LAST_RESULTS = res

    out = np.zeros((B, N, C), dtype=np.float32)
    for c in range(NCORES):
        out[c // 4] += np.asarray(res.results[c]["out"],
                                  dtype=np.float32)
    out += b_proj + b_qkv[2 * C:] @ w_proj
    return out


# revision 11
# speedup vs baseline: 1.0455x; 1.0455x over previous
"""Multi-head attention block on 8 TRN2 NeuronCores.

Problem: x[2,2048,768] -> qkv proj -> 12-head attention -> out proj.
Sharding: 24 (batch, head) pairs across 8 cores; core c handles batch
c//4 and heads 3*(c%4)..3*(c%4)+2. Each core computes its heads'
Q,K,V, attention, and a partial output projection; the host sums the
four per-batch partials and adds the bias terms.

v3 design notes:
  - All matmul operands are bf16 (1 cycle/row on the PE at any free
    size); PSUM accumulation stays f32; DRAM in/out tensors are bf16
    (host converts), halving DMA bytes vs f32.
  - Phase B is ACT-bound: 96 exps of [128,1024] (~1.05us effective
    spacing) are the kernel's pace-setter, so ACT does nothing else.
    Phase A work is spread: Q+K PSUM copies on DVE, V copies on
    gpsimd, pad zero-fills/ones via memsets (no DMA). The Exp table
    is preloaded via a dummy activation during the DMA wait.
  - Input DMA is split across two hardware queues (sync + gpsimd) in
    critical-first order so the first projection matmul waits only on
    ~1MB and phase A is never DMA-starved.
  - Phase A: per 512-column slab of x^T, one fused Q+K projection
    (3 PSUM groups of 128 = 384 rows; wq pre-scaled by 0.125
    host-side) plus keys-major V (192 free) for 4 key chunks.
  - Attention: S -> exp -> AV via a global pend queue (lag 2);
    denominators from a ones column appended to V (AV PSUM row 64);
    normalize = approx-reciprocal straight off the PSUM denominator
    row + one DVE multiply with a 0-stride partition-broadcast
    operand (no gpsimd broadcast, no sums staging).
  - Output projection: dedicated 2-bank PSUM tag; block 0's eight
    chunks ride the ACT-paced PE slack one-at-a-time every other
    m-round early in block 1 (DVE casts); block 1's chunks form the
    tail, alternating between the proj tag and the then-free S tag,
    with PSUM->SBUF casts on the then-idle ACT engine.
  - k-bias dropped (softmax shift invariance); v-bias and proj-bias
    fold in on the host: out += b_proj + b_v @ w_proj; q-bias adds on
    the DVE Q-copy (scaled 0.125 host-side like wq).
"""

import os
import sys

for _p in ("/opt/trn_rl_repo", "/opt/pypackages"):
    if _p not in sys.path:
        sys.path.append(_p)

import numpy as np

B, N, C = 2, 2048, 768
H, D = 12, 64
HPC = 3                    # heads per core
J = HPC * D                # 192 per-core head-dim rows
NCORES = 8
NBLK = 1024                # query-block width (one exp per [128, NBLK])
NB = N // NBLK             # 2
MC = N // 128              # 16 key chunks
KC = C // 128              # 6 contraction chunks for projections
NSLAB = 4                  # x^T column slabs of 512

_cache = {}
LAST_RESULTS = None


def _build():
    import concourse.mybir as mybir
    import concourse.tile as tile
    from concourse import bacc

    f32 = mybir.dt.float32
    bf16 = mybir.dt.bfloat16
    Exp = mybir.ActivationFunctionType.Exp
    Copy = mybir.ActivationFunctionType.Copy
    mult = mybir.AluOpType.mult
    add = mybir.AluOpType.add

    nc = bacc.Bacc("TRN2", target_bir_lowering=False, debug=False,
                   num_devices=NCORES)

    xt_d = nc.declare_dram_parameter("xt", [C, N], bf16, isOutput=False)
    wqk_d = nc.declare_dram_parameter("wqk", [C, 2 * J], bf16,
                                      isOutput=False)
    wv_d = nc.declare_dram_parameter("wv", [C, J], bf16, isOutput=False)
    bq_d = nc.declare_dram_parameter("bq", [J, 1], f32, isOutput=False)
    wp_d = nc.declare_dram_parameter("wp", [2 * 128, C], bf16,
                                     isOutput=False)
    out_d = nc.declare_dram_parameter("out", [N, C], bf16, isOutput=True)

    with tile.TileContext(nc) as tc:
        with (
            tc.tile_pool(name="persist", bufs=1) as pp,
            tc.tile_pool(name="osb", bufs=4) as posb,
        ):
            bqt = [pp.tile([64, 1], f32, tag=f"bq{h}", name=f"bq{h}")
                   for h in range(HPC)]
            xt = [pp.tile([128, N], bf16, tag=f"xt{i}", name=f"xt{i}")
                  for i in range(KC)]
            wqk = [pp.tile([128, 2 * J], bf16, tag=f"wqk{i}", name=f"wqk{i}")
                   for i in range(KC)]
            wv = [pp.tile([128, J], bf16, tag=f"wv{i}", name=f"wv{i}")
                  for i in range(KC)]
            # padded Q^T/K^T per head: rows 0:64 data, rows 64:128 zero
            qh = [pp.tile([128, N], bf16, tag=f"qh{h}", name=f"qh{h}")
                  for h in range(HPC)]
            kh = [pp.tile([128, N], bf16, tag=f"kh{h}", name=f"kh{h}")
                  for h in range(HPC)]
            # V keys-major with a ones column per head: [128, 3*65]
            vx = [pp.tile([128, HPC * 65], bf16, tag=f"vx{m}",
                          name=f"vx{m}") for m in range(MC)]
            wp = [pp.tile([128, C], bf16, tag=f"wp{t}", name=f"wp{t}")
                  for t in range(2)]
            # normalized attention outputs: ah2[0] = heads 0,1;
            # ah2[1] rows 0:64 = head 2, rows 64:128 zero
            ah2 = [pp.tile([128, N], bf16, tag=f"ah2{t}", name=f"ah2{t}")
                   for t in range(2)]
            dummy = pp.tile([1, 4], f32, tag="dummy", name="dummy")

            # gpsimd DMA queue: wv, xt slabs 2-3, wp (issued first so
            # the queue starts concurrently with sync's)
            for i in range(KC):
                nc.gpsimd.dma_start(wv[i][:], wv_d[128 * i:128 * (i + 1), :])
            for s in (2, 3):
                nsl = slice(512 * s, 512 * (s + 1))
                for i in range(KC):
                    nc.gpsimd.dma_start(xt[i][:, nsl],
                                        xt_d[128 * i:128 * (i + 1), nsl])
            for t in range(2):
                nc.gpsimd.dma_start(wp[t][:],
                                    wp_d[128 * t:128 * (t + 1), :])
            # sync DMA queue: bq, then wqk/xt-slab0 interleaved, slab1
            for h in range(HPC):
                nc.sync.dma_start(bqt[h][:], bq_d[64 * h:64 * (h + 1), :])
            for i in range(KC):
                nc.sync.dma_start(wqk[i][:],
                                  wqk_d[128 * i:128 * (i + 1), :])
                nc.sync.dma_start(xt[i][:, 0:512],
                                  xt_d[128 * i:128 * (i + 1), 0:512])
            for i in range(KC):
                nc.sync.dma_start(xt[i][:, 512:1024],
                                  xt_d[128 * i:128 * (i + 1), 512:1024])

            # zero-fill pads and ones columns; Exp table preload on ACT
            nc.vector.memset(dummy[:], 0.0)
            nc.scalar.activation(dummy[:, 2:4], dummy[:, 0:2], Exp)
            for h in range(HPC):
                nc.vector.memset(qh[h][64:128, :], 0.0)
                nc.gpsimd.memset(kh[h][64:128, :], 0.0)
            nc.vector.memset(ah2[1][64:128, :], 0.0)
            for m in range(MC):
                on = vx[m].rearrange("p (h e) -> p h e", e=65)[:, :, 64:65]
                nc.gpsimd.memset(on, 1.0)

            # ---- Phase A: fused Q+K projection + V, slab by slab ----
            with tc.tile_pool(name="ps1", bufs=2, space="PSUM") as ps1:
                for s in range(NSLAB):
                    nsl = slice(512 * s, 512 * (s + 1))
                    for g in range(3):
                        gsl = slice(128 * g, 128 * (g + 1))
                        ps = ps1.tile([128, 512], f32, tag="qk", bufs=3,
                                      name="ps_qk")
                        for k in range(KC):
                            nc.tensor.matmul(
                                ps[:], wqk[k][:, gsl], xt[k][:, nsl],
                                start=(k == 0), stop=(k == KC - 1))
                        for half in range(2):
                            idx = 2 * g + half
                            src = ps[64 * half:64 * (half + 1), :]
                            if idx < HPC:
                                nc.vector.tensor_scalar(
                                    qh[idx][0:64, nsl], src, 1.0,
                                    bqt[idx][:], mult, add)
                            else:
                                # ACT is idle in phase A; keeping K
                                # copies off DVE keeps the PSUM ring
                                # ahead of the PE
                                nc.scalar.activation(
                                    kh[idx - HPC][0:64, nsl], src, Copy)
                    for m in range(4 * s, 4 * s + 4):
                        msl = slice(128 * m, 128 * (m + 1))
                        psv = ps1.tile([128, J], f32, tag="v", bufs=2,
                                       name="ps_v")
                        for k in range(KC):
                            nc.tensor.matmul(psv[:], xt[k][:, msl], wv[k][:],
                                             start=(k == 0),
                                             stop=(k == KC - 1))
                        vdst = vx[m].rearrange("p (h e) -> p h e",
                                               e=65)[:, :, 0:64]
                        nc.vector.tensor_copy(
                            vdst, psv.rearrange("p (h e) -> p h e", e=64))

            # ---- Phase B: attention + interleaved projection ----
            with (
                tc.tile_pool(name="etile", bufs=4) as pe,
                tc.tile_pool(name="bcsb", bufs=4) as pbc,
                tc.tile_pool(name="ps2", bufs=1, space="PSUM") as ps2,
            ):
                pend = []

                def flush_one():
                    avh, h, nb, mm, ee = pend.pop(0)
                    vsl = slice(65 * h, 65 * (h + 1))
                    for i in range(NBLK // 512):
                        nc.tensor.matmul(
                            avh[i][:], vx[mm][:, vsl],
                            ee[:, 512 * i:512 * (i + 1)],
                            start=(mm == 0), stop=(mm == MC - 1))
                    if mm != MC - 1:
                        return
                    # normalize: approx-reciprocal of the PSUM
                    # denominator row, broadcast via 0-stride operand
                    adst, r0 = ((ah2[0], 0) if h == 0 else
                                (ah2[0], 64) if h == 1 else
                                (ah2[1], 0))
                    for i in range(NBLK // 512):
                        hf = slice(NBLK * nb + 512 * i,
                                   NBLK * nb + 512 * (i + 1))
                        dn = pbc.tile([1, 512], f32, tag="dn", name="dn")
                        nc.vector.tensor_copy(dn[:], avh[i][64:65, :])
                        rec = pbc.tile([1, 512], f32, tag="rec",
                                       name="rec")
                        nc.vector.reciprocal_approx_fast(rec[:], dn[:])
                        bcs = pbc.tile([64, 512], f32, tag="bcs",
                                       name="bcs")
                        nc.gpsimd.partition_broadcast(bcs[:], rec[:])
                        nc.vector.tensor_mul(
                            adst[r0:r0 + 64, hf], avh[i][0:64, :], bcs[:])

                def proj_chunk(mi, on_act):
                    msl = slice(128 * mi, 128 * (mi + 1))
                    pj = ps2.tile([128, NBLK], f32, tag="s", bufs=2,
                                  name="ps_pj")
                    for f0, fn in ((0, 512), (512, 256)):
                        for t in range(2):
                            nc.tensor.matmul(
                                pj[:, f0:f0 + fn], ah2[t][:, msl],
                                wp[t][:, f0:f0 + fn],
                                start=(t == 0), stop=(t == 1))
                    o3 = posb.tile([128, C], bf16, tag="o3", name="o3")
                    if on_act:
                        nc.scalar.activation(o3[:], pj[:, 0:C], Copy)
                    else:
                        nc.vector.tensor_copy(o3[:], pj[:, 0:C])
                    nc.sync.dma_start(out_d[msl, :], o3[:])

                for nb in range(NB):
                    for h in range(HPC):
                        avh = [ps2.tile([65, 512], f32, tag=f"av{i}",
                                        bufs=2, name=f"ps_av{i}")
                               for i in range(NBLK // 512)]
                        for m in range(MC):
                            msl = slice(128 * m, 128 * (m + 1))
                            s = ps2.tile([128, NBLK], f32, tag="s",
                                         bufs=2, name="ps_s")
                            for i in range(NBLK // 512):
                                nc.tensor.matmul(
                                    s[:, 512 * i:512 * (i + 1)],
                                    kh[h][:, msl],
                                    qh[h][:, NBLK * nb + 512 * i:
                                          NBLK * nb + 512 * (i + 1)])
                            e = pe.tile([128, NBLK], bf16, tag="e",
                                        name="e")
                            nc.scalar.activation(e[:], s[:], Exp)
                            pend.append((avh, h, nb, m, e))
                            if len(pend) > 2:
                                flush_one()
                            # block-0 projection rides the ACT-paced PE
                            # slack, one chunk every other m-round
                            if nb == 1 and h == 0 and m >= 2 and m % 2 == 0:
                                proj_chunk(m // 2 - 1, False)
                            if nb == 1 and h == 1 and m == 2:
                                proj_chunk(7, False)
                while pend:
                    flush_one()
                for mi in range(8, 16):
                    proj_chunk(mi, bool(mi % 2))

    nc.compile()
    return nc


def kernel(x, w_qkv, b_qkv, w_proj, b_proj):
    import ml_dtypes
    from concourse.bass_utils import run_bass_kernel_spmd

    global LAST_RESULTS
    if "nc" not in _cache:
        _cache["nc"] = _build()
    nc = _cache["nc"]

    bf = ml_dtypes.bfloat16
    x = np.asarray(x, dtype=np.float32)
    w_qkv = np.asarray(w_qkv, dtype=np.float32)
    b_qkv = np.asarray(b_qkv, dtype=np.float32)
    w_proj = np.asarray(w_proj, dtype=np.float32)
    b_proj = np.asarray(b_proj, dtype=np.float32)

    in_maps = []
    for c in range(NCORES):
        b = c // 4
        h0 = HPC * (c % 4)
        cs = slice(64 * h0, 64 * (h0 + HPC))
        ks = slice(C + 64 * h0, C + 64 * (h0 + HPC))
        vs = slice(2 * C + 64 * h0, 2 * C + 64 * (h0 + HPC))
        wqk_cat = np.concatenate(
            [w_qkv[:, cs] * 0.125, w_qkv[:, ks]], axis=1)
        wp_pad = np.zeros((2 * 128, C), dtype=np.float32)
        wp_pad[0:128] = w_proj[64 * h0:64 * (h0 + 2), :]
        wp_pad[128:192] = w_proj[64 * (h0 + 2):64 * (h0 + 3), :]
        in_maps.append({
            "xt": np.ascontiguousarray(x[b].T).astype(bf),
            "wqk": wqk_cat.astype(bf),
            "wv": np.ascontiguousarray(w_qkv[:, vs]).astype(bf),
            "bq": np.ascontiguousarray(
                (b_qkv[cs] * 0.125).reshape(J, 1)),
            "wp": wp_pad.astype(bf),
        })

    res = run_bass_kernel_spmd(nc, in_maps, core_ids=list(range(NCORES)))
    LAST_RESULTS = res

    out = np.zeros((B, N, C), dtype=np.float32)
    for c in range(NCORES):
        out[c // 4] += np.asarray(res.results[c]["out"],
                                  dtype=np.float32)
    out += b_proj + b_qkv[2 * C:] @ w_proj
    return out
